# revision 33
# baseline (speedup 1.0000x reference)
"""Trainium2 Bass kernel: cubic B-spline upsampling x2 of a (2,3,96,96,96) volume.

Math: the reference op (recursive IIR prefilter along each spatial axis, then
an 8-tap stride-2 transposed conv along each axis) is linear and separable.
The whole per-axis operator is a dense 192x96 matrix M (built exactly on the
host in float64).  out = M (x) M (x) M applied along z, y, x.

Device strategy (8 NeuronCores, SPMD, no collectives): each core computes a
z'-HALF (96 rows) of one volume ("A") plus a z'-QUARTER (48 rows) of another
("B") - 8x(96+48) = 6x192 output rows.  Because M's rows have ~exponentially
decaying support (prefilter pole^k), a 64-row z-slab of the input suffices for
either piece (truncation ~1e-7), so the core's whole input is ONE [128, 96x128]
bf16 tensor: partitions 0:64 = A-slab, 64:128 = B-slab (keeps every input DMA
at full 128-partition width; the pass-B matmuls use tile_position row 64).

Per piece, three matmul stages in data-stationary form (stationary operand =
data tile, moving operand = spline matrix) so NO transposes are needed:
  A: per x (96 mms): lhsT = vol[slab, x, y-pad128]   rhs = mz (64, 96|48) -> (y, z')
  B: per z' (N mms): lhsT = L1[y, z', x-pad128]      rhs = mt (128, 192)  -> (x, y')
  C: per 128-chunk of (z'y') : lhsT = L2f[x, chunk]  rhs = mt (128, 192)  -> (chunk, x')
Stage-C results stream to DRAM in a partition-major layout [128, 216, 192]
(fully contiguous DMA on both sides); the host unscrambles.  PSUM->SBUF copies
(the co-critical resource: DVE/ACT run them at ~1 elem/cycle) are uniform
768-element instructions alternated ACT:DVE = 5:4.  Emission order pipelines
the phases: A(volA); B/C(volA) interleaved; A(volB) + carried C(volA);
B/C(volB) + tail of C(volA), so the PE, both copy engines, and the out-DMA
stream all stay busy.  A burst of K=128 junk matmuls on mt right after the
entry barrier completes the HAM k=8/8 clock ramp (~4us of sustained
full-array activity; cold is ~1.7x slower) during the input-DMA wait.
Compute in bf16 (PSUM fp32); output bf16, upcast on the host (rel err
~5.3e-3 vs the reference).
"""

import math
import os
import sys

import numpy as np

for _p in ("/opt/trn_rl_repo",):
    if _p not in sys.path and os.path.isdir(_p):
        sys.path.insert(0, _p)

import ml_dtypes  # noqa: E402

BF16 = ml_dtypes.bfloat16

POLE = math.sqrt(3.0) - 2.0
GAIN = (1.0 - POLE) * (1.0 - 1.0 / POLE)  # 6.0
N = 96
F = 2
NOUT = N * F  # 192
NCORES = 8
SLAB = 64  # z-rows of input slab per piece (banded prefilter support)


def _cubic(t):
    a = np.abs(t)
    out = (2.0 / 3.0 + (0.5 * a - 1.0) * a**2) * (a < 1)
    out = out + (-((a - 2.0) ** 3) / 6.0) * ((a >= 1) & (a < 2))
    return out


def _prefilter_mat(n):
    """96x96 matrix of the causal+anticausal cubic-spline prefilter (float64)."""
    p = POLE
    xm = np.eye(n, dtype=np.float64) * GAIN
    i = np.arange(n)
    pows = p**i + p ** (2 * n - 1 - i)
    c = np.zeros((n, n), dtype=np.float64)
    c[0] = (pows @ xm) * (p / (1.0 - p ** (2 * n))) + xm[0]
    for k in range(1, n):
        c[k] = xm[k] + p * c[k - 1]
    out = np.zeros((n, n), dtype=np.float64)
    out[n - 1] = c[n - 1] * (p / (p - 1.0))
    for k in range(n - 2, -1, -1):
        out[k] = p * (out[k + 1] - c[k])
    return out


def _upsample_mat(n, f=F):
    """2n x n matrix of the edge-padded stride-2 transposed conv (float64)."""
    k = 4 * f  # f even -> is_odd == 0
    start = 1.0 / (2 * f) - 2.0
    pts = np.arange(k, dtype=np.float64) * (1.0 / f) + start
    ker = _cubic(pts)
    npad = n + 4
    U = np.zeros((f * n, npad), dtype=np.float64)
    for o in range(f * n):
        for i in range(npad):
            s = o + (k - 1) - f * i
            if 0 <= s < k:
                U[o, i] += ker[s]
    Uc = np.zeros((f * n, n), dtype=np.float64)
    for i in range(npad):
        j = min(max(i - 2, 0), n - 1)
        Uc[:, j] += U[:, i]
    return Uc


def build_M():
    """Exact 192x96 per-axis operator (float64)."""
    return _upsample_mat(N) @ _prefilter_mat(N)


def _slab_lo(M, r0, r1, width=SLAB):
    """Start of the width-row z-slab supporting output rows [r0, r1)."""
    sub = np.abs(M[r0:r1, :])
    mask = sub.max(axis=0) > sub.max() * 1e-7
    zlo = int(np.argmax(mask))
    zhi = N - int(np.argmax(mask[::-1]))
    assert zhi - zlo <= width, (r0, r1, zlo, zhi)
    lo = min(zlo, N - width)
    lo = max(lo, zhi - width)
    return lo


def _assign(core):
    """core -> (volA, halfA, volB, quarterB); vol in 0..5 = 3*b + c... (b,c)=divmod(vol,3)."""
    g, i = divmod(core, 4)
    return 3 * g + (i // 2), i % 2, 3 * g + 2, i


_NC_CACHE = {}


def _strip_redundant_self_waits(nc):
    """Drop sem waits that are trivially satisfied by same-engine program order.

    Tile's per-proc wait emission is not transitively minimal: a PE matmul can
    end up waiting on the PE's own semaphore (already guaranteed by in-order
    engine execution) in addition to a cross-engine wait, and the MM ISA
    struct only has one sync-wait slot (walrus: "Too many sync wait
    commands"). A wait on sem S is redundant for instruction I on engine E iff
    S is only ever updated by E and the cumulative updates to S from E before
    I already reach the wait value.
    """
    import concourse.mybir as mybir

    for fn in nc.m.functions:
        for blk in fn.blocks:
            updaters = {}  # sem id -> set of engines updating it (block-wide)
            for i in blk.instructions:
                si = i.sync_info
                if si is None:
                    continue
                for u in si.on_update or []:
                    updaters.setdefault(u.id, set()).add(i.engine)
            seen = {}  # (engine, sem id) -> cumulative update count so far
            for i in blk.instructions:
                si = i.sync_info
                if si is None:
                    continue
                if si.on_wait:
                    kept = []
                    for w in si.on_wait:
                        if (
                            w.sync_type == "semaphore"
                            and w.wait_mode == "sem-ge-imm"
                            and updaters.get(w.id) == {i.engine}
                            and seen.get((i.engine, w.id), 0) >= w.wait_value
                        ):
                            continue  # implied by program order
                        kept.append(w)
                    if len(kept) != len(si.on_wait):
                        si.on_wait[:] = kept
                for u in si.on_update or []:
                    key = (i.engine, u.id)
                    seen[key] = seen.get(key, 0) + u.update_value
            # each engine ISA struct has a single sync-wait slot: offload
            # extra waits onto same-engine nops inserted just before
            new_insts = []
            nop_n = 0
            for i in blk.instructions:
                si = i.sync_info
                if si is not None and si.on_wait and len(si.on_wait) > 1:
                    extra = list(si.on_wait[:-1])
                    si.on_wait[:] = [si.on_wait[-1]]
                    for w in extra:
                        nop = mybir.InstNoOp(
                            name=f"I-waitnop-{blk.name}-{nop_n}", ins=[], outs=[]
                        )
                        nop_n += 1
                        nop.engine = i.engine
                        nop.sync_info = mybir.SyncInfo(on_wait=[w], on_update=[])
                        new_insts.append(nop)
                new_insts.append(i)
            if nop_n:
                blk.instructions[:] = new_insts


def _dedup_a_ldweights(nc):
    """Mark the 2nd matmul of each identical-stationary pair as pre-loaded.

    Stage A emits two N=48 matmuls per x-slice with the SAME lhsT (z'-halves
    into different PSUM banks); walrus emits one LDWEIGHTS per matmul, so the
    stationary is loaded twice (~82ns each, the stage-A pacer). Setting
    `ldweights` on the second matmul of a back-to-back same-weights pair
    tells codegen the PE array already holds the weights.
    """
    for fn in nc.m.functions:
        for blk in fn.blocks:
            prev_key = None
            for i in blk.instructions:
                tn = type(i).__name__
                if tn == "InstMatmult":
                    key = str(i.ins[1])
                    if key == prev_key:
                        i.ldweights = True
                    prev_key = key
                elif tn in ("InstLdweights", "InstTranspose"):
                    prev_key = None


def _hoist_input_dmas(nc, n_hoist=3):
    """Move the first input DMAs ahead of the sync engine's entry barrier.

    The Tile/BSP prologue (entry EVSEM barrier + TENSOR_LOAD) takes ~6us of
    engine code-loading; the leading input DMAs have no waits (inputs are
    resident at NEFF start, dst tiles untouched), so issuing them in the
    prologue overlaps the transfer with the code load. Only ~0.5 MB is
    hoisted: the barrier waits for prologue DMAs to complete.
    """
    import concourse.mybir as mybir

    blocks = nc.m.functions[0].blocks
    body = blocks[1]
    dmas = []
    for i in body.instructions:
        if type(i).__name__ == "InstDMACopy" and i.engine == mybir.EngineType.SP:
            si = i.sync_info
            if si is not None and si.on_wait:
                break  # stop at the first gated DMA
            dmas.append(i)
            if len(dmas) >= n_hoist:
                break
    if not dmas:
        return
    dset = set(id(x) for x in dmas)
    body.instructions[:] = [i for i in body.instructions if id(i) not in dset]
    pro = blocks[0].instructions
    pos = 1 if pro and type(pro[0]).__name__ == "InstCall" else 0
    pro[:] = pro[:pos] + dmas + pro[pos:]


def build_nc():
    import concourse.bass as bass
    import concourse.mybir as mybir
    from concourse.tile import TileContext

    bf16 = mybir.dt.bfloat16
    f32 = mybir.dt.float32

    nc = bass.Bass(enable_partition_id=False)
    vol_ext = nc.declare_dram_parameter("vol", [128, 96 * 128], bf16, isOutput=False)
    mz_ext = nc.declare_dram_parameter("mz", [128, 144], bf16, isOutput=False)
    mt_ext = nc.declare_dram_parameter("mt", [128, 192], bf16, isOutput=False)
    out_ext = nc.declare_dram_parameter("out", [128, 216, 192], bf16, isOutput=True)

    with TileContext(nc) as tc:
        with (
            tc.tile_pool(name="consts", bufs=1) as consts,
            tc.tile_pool(name="vols", bufs=1) as vols_pool,
            tc.tile_pool(name="l1a", bufs=1) as l1a_pool,
            tc.tile_pool(name="l1b", bufs=1) as l1b_pool,
            tc.tile_pool(name="l2a", bufs=1) as l2a_pool,
            tc.tile_pool(name="l2b", bufs=1) as l2b_pool,
            tc.tile_pool(name="stage", bufs=6) as stage_pool,
            tc.tile_pool(name="pab", bufs=2, space="PSUM") as pab_pool,
            tc.tile_pool(name="pc", bufs=2, space="PSUM") as pc_pool,
        ):
            mt = consts.tile([128, 192], bf16)
            nc.sync.dma_start(out=mt[:], in_=mt_ext[:])
            mz = consts.tile([128, 144], bf16)
            nc.sync.dma_start(out=mz[:], in_=mz_ext[:])

            vol = vols_pool.tile([128, 96 * 128], bf16)
            # first chunk on sync (hoisted into the prologue: it overlaps
            # the fixed ~6us engine code-load and the entry barrier only has
            # to wait ~0.3 MB); the bulk goes via GpSimd SWDGE, whose Q7 is
            # otherwise idle -- HWDGE issue on sync costs ~700ns per DMA and
            # would delay the first out-DMAs behind 8 serial issues.
            sbounds = [0, 2048]
            for ch in range(1):
                nc.sync.dma_start(
                    out=vol[:, sbounds[ch] : sbounds[ch + 1]],
                    in_=vol_ext[:, sbounds[ch] : sbounds[ch + 1]],
                )
            gbounds = [2048, 4096, 6144, 8192, 10240, 12288]
            for ch in range(5):
                nc.gpsimd.dma_start(
                    out=vol[:, gbounds[ch] : gbounds[ch + 1]],
                    in_=vol_ext[:, gbounds[ch] : gbounds[ch + 1]],
                )

            L1A = l1a_pool.tile([128, 96, 128], bf16)
            L1B = l1b_pool.tile([128, 48, 128], bf16)
            L2A = l2a_pool.tile([128, 96, 192], bf16)
            L2B = l2b_pool.tile([128, 48, 192], bf16)
            nc.gpsimd.memset(L1A[:, :, 96:128], 0.0)
            nc.gpsimd.memset(L1B[:, :, 96:128], 0.0)

            # PSUM->SBUF copies: alternate ACT:DVE = 5:4 (ACT is ~1.2 vs
            # DVE ~0.96 elem/ns from PSUM); all copies are 768 elems/part.
            cp_state = {"i": 0}

            def cpy(dst, src):
                k = cp_state["i"] % 9
                cp_state["i"] += 1
                if k % 2 == 0:
                    nc.scalar.copy(dst, src)
                else:
                    nc.vector.tensor_copy(dst, src)

            # ---- stage C emission machinery ----
            # `fine` is a list of chunk-counts for tapered tail tiles (each
            # gets its own small DMA so the final drain is short); normal
            # tiles are 4 chunks, DMA'd in 12-chunk stage tiles.
            def make_cemit(L2f, n_tiles, col_base, fine=(), stage_n=3):
                st = {"t": 0, "ch": 0, "stage": None}
                normal = n_tiles - len(fine)

                def emit():
                    t = st["t"]
                    if t < normal:
                        gi = t % stage_n
                        if gi == 0:
                            st["stage"] = stage_pool.tile(
                                [128, 4 * stage_n, 192], bf16, name="stg"
                            )
                        stg = st["stage"]
                        nch = 4
                        last = gi == stage_n - 1
                    else:
                        gi = 0
                        nch = fine[t - normal]
                        stg = stage_pool.tile([128, nch, 192], bf16, name="stgf")
                        last = True
                    pc = pc_pool.tile(
                        [128, 4, 192], f32, padded_shape=[128, 4, 256], name="pc"
                    )
                    c0 = st["ch"]
                    for j in range(nch):
                        nc.tensor.matmul(
                            pc[:, j, :],
                            lhsT=L2f[:, (c0 + j) * 128 : (c0 + j + 1) * 128],
                            rhs=mt[:],
                            start=True,
                            stop=True,
                        )
                    cpy(stg[:, gi * 4 : gi * 4 + nch, :], pc[:, 0:nch, :])
                    if last:
                        d0 = col_base + c0 + nch - (gi * 4 + nch)
                        nc.sync.dma_start(
                            out=out_ext[:, d0 : d0 + gi * 4 + nch, :],
                            in_=stg[:, 0 : gi * 4 + nch, :],
                        )
                    st["t"] += 1
                    st["ch"] += nch

                return emit, st

            # ---- P1: stage A for volA (z'-half, 96 z'-rows) ----
            # PSUM matmul writes must keep >=32B element stride (16B-stride
            # writes measured ~2ns/elem) and stay within one 2KB bank, so
            # each x-slice does two N=48 matmuls (z'-halves, one per bank).
            # All stage-A matmuls are K=128 (the other slab's mz rows are
            # zero): a K=64 matmul only powers half the PE array and the HAM
            # power manager never ramps the clock (k=4/8 is ~1.7x slower).
            # PE warm-up: K=128 junk matmuls on mt right after it loads
            # (~7us), so the HAM k=8/8 clock ramp (~3-4us of sustained
            # full-array activity) completes during the input-DMA wait
            # instead of eating the first ~4us of stage A at ~1.7x. The
            # results are never read; the pool reuses the banks for stage C.
            for w in range(4):
                pj = pc_pool.tile(
                    [128, 4, 192], f32, padded_shape=[128, 4, 256], name="pc"
                )
                for j in range(4):
                    nc.tensor.matmul(
                        pj[:, j, :],
                        lhsT=mt[:, 0:128],
                        rhs=mt[:],
                        start=True,
                        stop=True,
                    )

            # j-major PSUM layout: one N=96 matmul per x-slice with a fully
            # CONTIGUOUS 384B out run (one LDWEIGHTS+MATMUL instead of two);
            # the copy pays the transpose with a 512B-strided PSUM read,
            # which lands in P1's otherwise-idle copy-engine window.
            for g in range(12):
                pa = pab_pool.tile(
                    [128, 8, 128], f32, name="pa", tag="pab"
                )
                for j in range(8):
                    x = g * 8 + j
                    nc.tensor.matmul(
                        pa[:, j, 0:96],
                        lhsT=vol[:, x * 128 : (x + 1) * 128],
                        rhs=mz[:, 0:96],
                        start=True,
                        stop=True,
                    )
                cpy(
                    L1A[:, :, g * 8 : (g + 1) * 8],
                    pa[:, :, 0:96].rearrange("p j z -> p z j"),
                )

            # ---- P2: stage B volA + paced stage C volA ----
            L2Af = L2A[:].rearrange("p a b -> p (a b)")  # (128, 18432)
            emit_a, st_a = make_cemit(L2Af, 36, 0, stage_n=6)
            for zz in range(24):
                pb = pab_pool.tile(
                    [128, 2, 2, 192], f32, name="pb", tag="pab",
                    padded_shape=[128, 2, 2, 256],
                )
                for b2 in range(2):
                    for jj in range(2):
                        zp = zz * 4 + b2 * 2 + jj
                        nc.tensor.matmul(
                            pb[:, b2, jj, :],
                            lhsT=L1A[:, zp, :],
                            rhs=mt[:],
                            start=True,
                            stop=True,
                        )
                cpy(
                    L2A[:, zz * 4 : zz * 4 + 4, :].rearrange(
                        "p (b j) y -> p b j y", b=2
                    ),
                    pb[:, :, :, :],
                )
                avail = 3 * (zz + 1) // 2
                pace = (27 * (zz + 1) + 23) // 24
                while st_a["t"] < min(avail, pace):
                    emit_a()

            # ---- P3: stage A for volB (z'-quarter) + carried C volA ----
            for g in range(6):
                pab2 = pab_pool.tile(
                    [128, 2, 64, 8], f32, name="pab2", tag="pab"
                )
                for h in range(2):
                    for j in range(8):
                        x = g * 16 + h * 8 + j
                        nc.tensor.matmul(
                            pab2[:, h, 0:48, j],
                            lhsT=vol[:, x * 128 : (x + 1) * 128],
                            rhs=mz[:, 96:144],
                            start=True,
                            stop=True,
                        )
                cpy(
                    L1B[:, :, g * 16 : (g + 1) * 16].rearrange(
                        "p z (h j) -> p h z j", h=2
                    ),
                    pab2[:, :, 0:48, :],
                )
                while st_a["t"] < 27 + g + 1:
                    emit_a()

            # ---- P4: drain C volA, then stage B volB + paced C volB ----
            while st_a["t"] < 36:
                emit_a()
            L2Bf = L2B[:].rearrange("p a b -> p (a b)")  # (128, 9216)
            emit_b, st_b = make_cemit(L2Bf, 20, 144, (4, 4, 2, 1, 1))
            for zz in range(12):
                pb = pab_pool.tile(
                    [128, 2, 2, 192], f32, name="pb", tag="pab",
                    padded_shape=[128, 2, 2, 256],
                )
                for b2 in range(2):
                    for jj in range(2):
                        zp = zz * 4 + b2 * 2 + jj
                        nc.tensor.matmul(
                            pb[:, b2, jj, :],
                            lhsT=L1B[:, zp, :],
                            rhs=mt[:],
                            start=True,
                            stop=True,
                        )
                cpy(
                    L2B[:, zz * 4 : zz * 4 + 4, :].rearrange(
                        "p (b j) y -> p b j y", b=2
                    ),
                    pb[:, :, :, :],
                )
                while st_b["t"] < 20 and st_b["ch"] + 4 <= 6 * (zz + 1):
                    emit_b()
            while st_b["t"] < 20:
                emit_b()
    _dedup_a_ldweights(nc)
    _strip_redundant_self_waits(nc)
    _hoist_input_dmas(nc)
    return nc


def make_in_maps(volume, M):
    mt_b = np.zeros((128, 192), dtype=BF16)  # K-padded to 128 rows
    mt_b[:96] = np.ascontiguousarray(M.T).astype(BF16)
    in_maps = []
    for core in range(NCORES):
        vA, hA, vB, qB = _assign(core)
        bA, cA = divmod(vA, 3)
        bB, cB = divmod(vB, 3)
        loA = _slab_lo(M, 96 * hA, 96 * hA + 96)
        loB = _slab_lo(M, 48 * qB, 48 * qB + 48)
        vol_host = np.zeros((128, 96, 128), dtype=BF16)
        vtA = np.transpose(volume[bA, cA], (0, 2, 1))  # (z, x, y)
        vtB = np.transpose(volume[bB, cB], (0, 2, 1))
        vol_host[0:64, :, :96] = vtA[loA : loA + SLAB].astype(BF16)
        vol_host[64:128, :, :96] = vtB[loB : loB + SLAB].astype(BF16)
        mz = np.zeros((128, 144), dtype=BF16)
        mz[0:64, 0:96] = np.ascontiguousarray(
            M[96 * hA : 96 * hA + 96, loA : loA + SLAB].T
        ).astype(BF16)
        mz[64:128, 96:144] = np.ascontiguousarray(
            M[48 * qB : 48 * qB + 48, loB : loB + SLAB].T
        ).astype(BF16)
        in_maps.append(
            {"vol": vol_host.reshape(128, 96 * 128), "mz": mz, "mt": mt_b}
        )
    return in_maps


def gather_out(results):
    out = np.zeros((2, 3, 192, 192, 192), dtype=np.float32)
    for core in range(NCORES):
        vA, hA, vB, qB = _assign(core)
        bA, cA = divmod(vA, 3)
        bB, cB = divmod(vB, 3)
        o = np.asarray(results[core]["out"], dtype=np.float32)  # [128, 216, 192]
        oa = o[:, 0:144, :].transpose(1, 0, 2).reshape(96, 192, 192)
        out[bA, cA, 96 * hA : 96 * hA + 96] = oa
        ob = o[:, 144:216, :].transpose(1, 0, 2).reshape(48, 192, 192)
        out[bB, cB, 48 * qB : 48 * qB + 48] = ob
    return out


def run(volume, trace=False):
    """Returns (output, exec_time_ns_or_None)."""
    import concourse.bass_utils as bu
    from concourse.bass_utils import run_bass_kernel_spmd

    if trace:
        # avoid the S3 artifact upload in the axon trace path
        bu.upload_artifacts = lambda tmpdir: str(tmpdir)
        # shim antenv.axon_hooks if the image lacks it (the hook itself
        # lives in trn_agent_boot; only the antenv indirection is missing)
        try:
            from antenv.axon_hooks import get_axon_ntff_profile_hook  # noqa
        except ImportError:
            import types

            from trn_agent_boot.trn_boot import _ntff_profile_via_ctypes

            _hook = _ntff_profile_via_ctypes("/opt/axon/libaxon_pjrt.so")
            _mod = types.ModuleType("antenv.axon_hooks")
            _mod.get_axon_ntff_profile_hook = lambda: _hook
            sys.modules["antenv.axon_hooks"] = _mod

    volume = np.asarray(volume, dtype=np.float32)
    M = build_M()
    in_maps = make_in_maps(volume, M)
    if "nc" not in _NC_CACHE:
        _NC_CACHE["nc"] = build_nc()
    nc = _NC_CACHE["nc"]
    res = run_bass_kernel_spmd(
        nc, in_maps, core_ids=list(range(NCORES)), trace=trace
    )
    out = gather_out(res.results)
    return out, getattr(res, "exec_time_ns", None)


def kernel(volume):
    out, _ = run(volume, trace=False)
    return out
